# revision 12
# baseline (speedup 1.0000x reference)
"""GCN layer kernel for Trainium2, 8-core SPMD.

Computes: out = (A @ (X @ W + b)) / colsum(A)[:, None],  A = (adj != 0)
with N=8192 nodes, F_in=F_out=512, across 8 NeuronCores.

Sharding: row-shard adjacency and node features (1024 rows per core),
replicate W/b. Degree (column sums of A) needs rows from every core: each
core computes partial column sums for free via the binarize op's
accumulate output, an all-gather shares them, and an on-device tree sum
+ reciprocal finishes the normalization.

All tensor inputs are shipped to HBM as bf16 (host-side dtype cast only:
the (adj != 0) predicate is exactly preserved by the cast since no
uniform[0,1) float rounds to 0 in bf16, and X/W/b are converted to bf16
on-device by the reference-equivalent compute path anyway). This halves
the dominant A-stream HBM traffic (32 -> 16 MB/core). Binarize + degree
+ projection + aggregation + normalization all still happen on device.

Measured-on-hw notes driving the structure:
- bf16 [128,512] matmuls: 454ns latency, ~265ns pipelined throughput
  (~1.95GHz effective). Main loop floor = 512 mm ~= 136us.
- The CC engine cannot enter a collective before ~20.5us, and the FIRST
  collective pays a ~50us setup; later ones run at wire speed. A tiny
  warm-up AllGather is issued first to absorb the setup, then the H
  exchange runs as TWO half-row AllGathers so the first half of every
  rank's H lands ~20us earlier than a monolithic gather would.
- Each core computes H = X@W+b for itself and the next LR-1 ranks; the
  main loop covers the gather window with rotated local iterations
  first, then visits far tiles sub-major (all ranks' rows 0:512 before
  any rows 512:1024) to consume the half-gathers as they land.
- The degree partials are accumulated at canonical rank-major columns
  (static APs) so the cross-rank combine stays two strided DMAs per
  rank regardless of iteration order.
- bias is added by one K=1 matmul into a broadcast tile at startup plus
  a Pool-engine tensor_tensor on each H evacuation (instead of LR*8
  extra matmuls); the DVE keeps only binarizes.
"""
import numpy as np

N = 8192
F = 512
N_CORES = 8
NB = N // N_CORES          # 1024 rows per core
KT = N // 128              # 64 contraction tiles
MT = NB // 128             # 8 output row tiles per core
FI_T = F // 128            # 4 feat-in tiles
LR = 3                     # ranks whose H we compute locally
FR = N_CORES - LR          # far ranks
RBUF = 5                   # a_raw ring depth (DMA run-ahead)
ABUF = 14                  # a_bin ring depth (DVE run-ahead)
HBUF = 16                  # gathered-H tile ring depth
PRE_BIN = 2                # binarizes interleaved per H rank block

_cached = {}


def _iter_order():
    """(r_off, sub) pairs: local ranks rank-major, then far tiles
    sub-major so the first half-gather covers iterations 24..23+4*FR."""
    order = []
    for r_off in range(LR):
        for sub in range(MT):
            order.append((r_off, sub))
    for sub in range(MT):
        for r_off in range(LR, N_CORES):
            order.append((r_off, sub))
    return order


def _build():
    import concourse.bacc as bacc
    import concourse.bass as bass
    import concourse.tile as tile
    from concourse import mybir

    f32 = mybir.dt.float32
    bf16 = mybir.dt.bfloat16

    nc = bacc.Bacc("TRN2", target_bir_lowering=False, debug=False,
                   num_devices=N_CORES)
    at = nc.dram_tensor("at", [N, NB], bf16, kind="ExternalInput").ap()
    xt = nc.dram_tensor("xt", [F, LR * NB], bf16, kind="ExternalInput").ap()
    w = nc.dram_tensor("w", [F, F], bf16, kind="ExternalInput").ap()
    bvec = nc.dram_tensor("bvec", [1, F], bf16, kind="ExternalInput").ap()
    out = nc.dram_tensor("out", [NB, F], f32, kind="ExternalOutput").ap()

    pid = nc.partition_id()
    HB2 = NB // 2

    with tile.TileContext(nc) as tc:
        with tc.tile_pool(name="dram", bufs=1, space="DRAM") as dram, \
             tc.tile_pool(name="p", bufs=1) as p, \
             tc.tile_pool(name="ps", bufs=1, space="PSUM") as ps:
            wu_in = dram.tile([1, 8], f32)
            wu_out = dram.tile([N_CORES, 8], f32, addr_space="Shared")
            hg_in_a = dram.tile([HB2, F], bf16)
            hg_in_b = dram.tile([HB2, F], bf16)
            hg_out_a = dram.tile([N // 2, F], bf16, addr_space="Shared")
            hg_out_b = dram.tile([N // 2, F], bf16, addr_space="Shared")
            dg_in = dram.tile([128, KT], f32)
            dg_out = dram.tile([128 * N_CORES, KT], f32, addr_space="Shared")

            # ---- warm-up collective: absorbs the first-collective setup
            # cost (~50us) so the H half-gathers run at wire speed.
            nc.gpsimd.collective_compute(
                "AllGather", mybir.AluOpType.bypass,
                replica_groups=[list(range(N_CORES))],
                ins=[wu_in.opt()], outs=[wu_out.opt()],
            )

            # ---- critical-path DMAs first: b, W chunk 0, rank-0 X chunks
            b_bf = p.tile([1, F], bf16)
            nc.scalar.dma_start(b_bf[:], bvec)
            w_c = p.tile([128, FI_T * F], bf16)
            nc.scalar.dma_start(w_c[:, 0:F], w[0:128, :])
            xtc_all = {}
            for ki in range(FI_T):
                xtc = p.tile([128, NB], bf16, tag="xtc",
                             bufs=LR * FI_T, name=f"xtc0_{ki}")
                nc.scalar.dma_start(xtc[:], xt[ki * 128:(ki + 1) * 128, 0:NB])
                xtc_all[(0, ki)] = xtc
            for ki in range(1, FI_T):
                nc.scalar.dma_start(w_c[:, ki * F:(ki + 1) * F],
                                    w[ki * 128:(ki + 1) * 128, :])
            for rr in range(1, LR):
                for ki in range(FI_T):
                    xtc = p.tile([128, NB], bf16, tag="xtc",
                                 bufs=LR * FI_T, name=f"xtc{rr}_{ki}")
                    nc.scalar.dma_start(
                        xtc[:],
                        xt[ki * 128:(ki + 1) * 128, rr * NB:(rr + 1) * NB])
                    xtc_all[(rr, ki)] = xtc

            cs = p.tile([128, KT], f32)    # per-core partial column sums
            ones = p.tile([128, NB], bf16)
            nc.vector.memset(ones[:], 1.0)
            ones1 = p.tile([1, 128], bf16)
            nc.vector.memset(ones1[:], 1.0)

            # single PSUM pool: 8 banks, all held by the main accumulators;
            # H compute / bias broadcast reuse them as scratch (the first
            # real matmul's start=True clears each bank).
            pms = []
            for m in range(MT):
                pm = ps.tile([128, F], f32, tag=f"pm{m}", name=f"pm{m}",
                             bufs=1)
                pms.append(pm)

            # bias broadcast [128, F] via one K=1 matmul
            nc.tensor.matmul(pms[7][:], ones1[:], b_bf[:],
                             start=True, stop=True)
            b_bcast = p.tile([128, F], f32)
            nc.vector.tensor_copy(b_bcast[:], pms[7][:])

            order = _iter_order()

            # ---- A stream ----
            a_raws = []

            def emit_araw(i):
                r_off, sub = order[i]
                a_raw = p.tile([128, NB], bf16, tag="araw", bufs=RBUF,
                               name=f"araw{i}")
                kt_e = (pid * MT + r_off * MT + sub) % KT
                nc.sync.dma_start(a_raw[:], at[bass.ds(kt_e * 128, 128), :])
                a_raws.append(a_raw)

            for i in range(RBUF):
                emit_araw(i)

            a_bins = []

            def emit_binarize(i):
                # one DVE op: a_bin = (a_raw != 0) * 1.0 (bf16, exact),
                # accum_out = free-dim sums = partial column sums of A,
                # stored at canonical rank-major column r_off*8+sub
                # (static AP regardless of iteration order).
                r_off, sub = order[i]
                a_bin = p.tile([128, NB], bf16, tag="abin", bufs=ABUF,
                               name=f"abin{i}")
                c = r_off * MT + sub
                nc.vector.scalar_tensor_tensor(
                    a_bin[:], a_raws[i][:], 0.0, ones[:],
                    mybir.AluOpType.not_equal, mybir.AluOpType.mult,
                    accum_out=cs[:, c:c + 1])
                a_bins.append(a_bin)

            # ---- H blocks for ranks pid..pid+LR-1 (bf16 matmuls) ----
            # evacuation adds the bias broadcast on the Pool engine.
            hb_all = []
            for rr in range(LR):
                for nt in range(MT):
                    hp = pms[nt % 4]
                    for ki in range(FI_T):
                        nc.tensor.matmul(
                            hp[:],
                            xtc_all[(rr, ki)][:, nt * 128:(nt + 1) * 128],
                            w_c[:, ki * F:(ki + 1) * F],
                            start=(ki == 0), stop=(ki == FI_T - 1))
                    hb = p.tile([128, F], bf16, tag="hb", bufs=LR * MT,
                                name=f"hb{rr}_{nt}")
                    nc.vector.tensor_tensor(hb[:], hp[:], b_bcast[:],
                                            mybir.AluOpType.add)
                    if rr == 0:
                        if nt < 4:
                            nc.sync.dma_start(
                                hg_in_a[nt * 128:(nt + 1) * 128, :], hb[:])
                        else:
                            nc.sync.dma_start(
                                hg_in_b[(nt - 4) * 128:(nt - 3) * 128, :],
                                hb[:])
                    hb_all.append(hb)
                if rr == 0:
                    # two half-gathers: the first can start as soon as our
                    # rows 0:512 are in hg_in_a and completes ~20us before
                    # a monolithic gather would.
                    nc.gpsimd.collective_compute(
                        "AllGather", mybir.AluOpType.bypass,
                        replica_groups=[list(range(N_CORES))],
                        ins=[hg_in_a.opt()], outs=[hg_out_a.opt()],
                    )
                    nc.gpsimd.collective_compute(
                        "AllGather", mybir.AluOpType.bypass,
                        replica_groups=[list(range(N_CORES))],
                        ins=[hg_in_b.opt()], outs=[hg_out_b.opt()],
                    )
                # keep the DVE fed with early binarizes between H blocks
                for _ in range(PRE_BIN):
                    i = len(a_bins)
                    if i < KT:
                        emit_binarize(i)
                        if len(a_raws) < KT:
                            emit_araw(len(a_raws))

            # Main loop: local iterations first (rank-major over ranks
            # pid..pid+LR-1), then far tiles sub-major. PSUM accumulation
            # is commutative.
            for i in range(KT):
                r_off, sub = order[i]
                if i >= len(a_bins):
                    emit_binarize(i)
                    if len(a_raws) < KT:
                        emit_araw(len(a_raws))
                if i < LR * MT:
                    rhs = hb_all[i][:]
                else:
                    r_abs = (pid + r_off) % N_CORES
                    h_t = p.tile([128, F], bf16, tag="ht", bufs=HBUF,
                                 name=f"ht{i}")
                    if sub < 4:
                        src = hg_out_a[bass.ds(r_abs * HB2 + sub * 128,
                                               128), :]
                    else:
                        src = hg_out_b[bass.ds(r_abs * HB2 + (sub - 4) * 128,
                                               128), :]
                    nc.scalar.dma_start(h_t[:], src)
                    rhs = h_t[:]
                for m in range(MT):
                    nc.tensor.matmul(
                        pms[m][:],
                        a_bins[i][:, m * 128:(m + 1) * 128],
                        rhs,
                        start=(i == 0), stop=(i == KT - 1))

            # ---- phase 3: degree + normalize ----
            nc.sync.dma_start(dg_in[:], cs[:])
            nc.gpsimd.collective_compute(
                "AllGather", mybir.AluOpType.bypass,
                replica_groups=[list(range(N_CORES))],
                ins=[dg_in.opt()], outs=[dg_out.opt()],
            )
            # cs columns are canonical rank-major: rank r's slab holds OUR
            # block (kt = pid*8 + m) at columns ((pid - r) mod 8)*8 + m.
            deg = p.tile([128, MT], f32)
            prt0 = p.tile([128, MT], f32, tag="prt", bufs=4, name="prt0")
            nc.gpsimd.dma_start(prt0[:], dg_out[0:128, bass.ts(pid, MT)])
            nc.vector.tensor_copy(deg[:], prt0[:])
            for r in range(1, N_CORES):
                col = ((pid + (N_CORES - r)) % N_CORES) * MT
                prt = p.tile([128, MT], f32, tag="prt", bufs=4,
                             name=f"prt{r}")
                nc.gpsimd.dma_start(
                    prt[:],
                    dg_out[r * 128:(r + 1) * 128, bass.ds(col, MT)])
                nc.vector.tensor_tensor(deg[:], deg[:], prt[:],
                                        mybir.AluOpType.add)
            rdeg = p.tile([128, MT], f32)
            nc.vector.reciprocal(rdeg[:], deg[:])

            for m in range(MT):
                o_sb = p.tile([128, F], f32, tag="osb", bufs=4,
                              name=f"osb{m}")
                if m % 2 == 0:
                    nc.vector.tensor_scalar(o_sb[:], pms[m][:],
                                            rdeg[:, m:m + 1], None,
                                            mybir.AluOpType.mult)
                else:
                    nc.scalar.mul(o_sb[:], pms[m][:], rdeg[:, m:m + 1])
                if m % 2 == 0:
                    nc.sync.dma_start(out[m * 128:(m + 1) * 128, :], o_sb[:])
                else:
                    nc.gpsimd.dma_start(out[m * 128:(m + 1) * 128, :],
                                        o_sb[:])

    nc.compile()
    return nc


def _get_nc():
    if "nc" not in _cached:
        _cached["nc"] = _build()
    return _cached["nc"]


def kernel(input_features, adj, W, b):
    import ml_dtypes
    from concourse.bass_utils import run_bass_kernel_spmd

    bf16 = ml_dtypes.bfloat16
    x = np.asarray(input_features, dtype=np.float32)
    a = np.asarray(adj, dtype=np.float32)
    wm = np.ascontiguousarray(np.asarray(W, dtype=np.float32).astype(bf16))
    bv = np.ascontiguousarray(
        np.asarray(b, dtype=np.float32).astype(bf16).reshape(1, F))

    xts = [np.ascontiguousarray(x[k * NB:(k + 1) * NB, :].T.astype(bf16))
           for k in range(N_CORES)]

    nc = _get_nc()
    in_maps = []
    for k in range(N_CORES):
        blk = slice(k * NB, (k + 1) * NB)
        xt_cat = np.concatenate(
            [xts[(k + rr) % N_CORES] for rr in range(LR)], axis=1)
        in_maps.append({
            "at": np.ascontiguousarray(a[blk, :].T.astype(bf16)),
            "xt": np.ascontiguousarray(xt_cat),
            "w": wm,
            "bvec": bv,
        })
    res = run_bass_kernel_spmd(nc, in_maps, core_ids=list(range(N_CORES)))
    return np.concatenate([res.results[k]["out"] for k in range(N_CORES)],
                          axis=0)
